# revision 1
# baseline (speedup 1.0000x reference)
"""Trainium2 Bass kernel for nn_CoreAttention (S=2048, B=1, H=16, D=128).

Sharding: 16 heads across 8 NeuronCores (2 heads/core, tensor parallel).

Per head (big tensors stay feature-major so nothing large is transposed
on device; the host supplies Q^T/K^T/V^T per head):
    qT     = (Wqk^T Q^T) / NF            (bf16 PE, fp32 PSUM)
    kT     = Wqk^T K^T                   (bf16 PE)
    scoresT[k,q] = kT-block^T @ qT       (bf16 PE; causal: only q >= k)
    scoresT += causal mask on diag block (PE accumulate of -1e4 tile)
    expT   = exp(scoresT)                (ACT, PSUM->SBUF bf16)
    sums[q]= ones-matmuls over expT      (PE, N=1 column sums)
    v      = V^T-chunks^T @ Wv           (bf16 PE -> natural [s,e] layout)
    ctxT   = sum_j v_j^T @ expT_j        (bf16 PE, fp32 accum)
    ctx    = transpose(ctxT) * (1/sums)  (fp32 PE transpose + DVE scale)

exp() runs without max-subtraction: scores are ~N(0,1) (the reference
normalizes by sqrt(128)), so exp never overflows and matches the
reference's masked softmax to rounding error.
"""

import sys
from contextlib import ExitStack

import numpy as np

for _p in ("/opt/trn_rl_repo",):
    if _p not in sys.path:
        sys.path.insert(0, _p)

import ml_dtypes
import concourse.bass as bass
import concourse.tile as tile
from concourse import bacc, mybir
from concourse.bass_utils import run_bass_kernel_spmd

S, B, H, D = 2048, 1, 16, 128
HPC = 2  # heads per core
NCORES = 8
NB = S // 128  # 16 seq blocks of 128
NF = float(np.sqrt(2048.0 / 16.0))  # NORM_FACTOR
NEG = -10000.0
PAD = 384  # zero-pad columns in front of each expt_j buffer

F32 = mybir.dt.float32
BF16 = mybir.dt.bfloat16
AF = mybir.ActivationFunctionType


def build_program() -> bass.Bass:
    nc = bacc.Bacc(
        "TRN2", target_bir_lowering=False, debug=False, num_devices=NCORES
    )

    qt_d = nc.dram_tensor("qt", [HPC, D, S], F32, kind="ExternalInput")
    kt_d = nc.dram_tensor("kt", [HPC, D, S], F32, kind="ExternalInput")
    vt_d = nc.dram_tensor("vt", [HPC, D, S], F32, kind="ExternalInput")
    wqk_d = nc.dram_tensor("wqk", [HPC, D, D], F32, kind="ExternalInput")
    wv_d = nc.dram_tensor("wv", [HPC, D, D], F32, kind="ExternalInput")
    identf_d = nc.dram_tensor("identf", [D, D], F32, kind="ExternalInput")
    identb_d = nc.dram_tensor("identb", [D, D], BF16, kind="ExternalInput")
    maskb_d = nc.dram_tensor("maskb", [D, D], BF16, kind="ExternalInput")
    onesb_d = nc.dram_tensor("onesb", [D, 1], BF16, kind="ExternalInput")
    onesf_d = nc.dram_tensor("onesf", [1, 1], F32, kind="ExternalInput")
    out_d = nc.dram_tensor("out", [HPC, S, D], F32, kind="ExternalOutput")

    with tile.TileContext(nc) as tc, ExitStack() as ctx:
        cpool = ctx.enter_context(tc.tile_pool(name="const", bufs=1))
        sb = ctx.enter_context(tc.tile_pool(name="sb", bufs=1))
        ps = ctx.enter_context(tc.tile_pool(name="ps", bufs=1, space="PSUM"))

        identf = cpool.tile([D, D], F32)
        nc.sync.dma_start(identf[:], identf_d[:])
        identb = cpool.tile([D, D], BF16)
        nc.sync.dma_start(identb[:], identb_d[:])
        maskb = cpool.tile([D, D], BF16)
        nc.sync.dma_start(maskb[:], maskb_d[:])
        onesb = cpool.tile([D, 1], BF16)
        nc.sync.dma_start(onesb[:], onesb_d[:])
        onesf = cpool.tile([1, 1], F32)
        nc.sync.dma_start(onesf[:], onesf_d[:])

        # Warm the PE's view of identf's DMA queue so later fp32 transposes
        # (self-loading, max 1 sync wait) never need a second wait.
        warm_ps = ps.tile([D, D], F32, tag="otr", name="warm_ps")
        nc.tensor.transpose(warm_ps[:], identf[:], identf[:])

        for h in range(HPC):
            # ---- load raw inputs (weights first: tiny, unblock projs) -----
            wqk = sb.tile([D, D], F32, tag="wqk", bufs=2)
            nc.sync.dma_start(wqk[:], wqk_d[h])
            wv = sb.tile([D, D], F32, tag="wv", bufs=2)
            nc.sync.dma_start(wv[:], wv_d[h])
            wqkb = sb.tile([D, D], BF16, tag="wqkb", bufs=2)
            nc.vector.tensor_copy(wqkb[:], wqk[:])
            wvb = sb.tile([D, D], BF16, tag="wvb", bufs=2)
            nc.vector.tensor_copy(wvb[:], wv[:])

            # q/k/v loads + bf16 casts, pipelined at 1024-col granularity
            qtr = sb.tile([D, S], F32, tag="qtr", bufs=2)
            ktr = sb.tile([D, S], F32, tag="ktr", bufs=2)
            vtr = sb.tile([D, S], F32, tag="vtr", bufs=2)
            qtb = sb.tile([D, S], BF16, tag="qtb", bufs=1)
            ktb = sb.tile([D, S], BF16, tag="ktb", bufs=1)
            vtb = sb.tile([D, S], BF16, tag="vtb", bufs=1)
            for raw, dr, cast in ((qtr, qt_d, qtb), (ktr, kt_d, ktb), (vtr, vt_d, vtb)):
                for c in range(2):
                    sl = slice(c * 1024, (c + 1) * 1024)
                    nc.sync.dma_start(raw[:, sl], dr[h][:, sl])
                    nc.vector.tensor_copy(cast[:, sl], raw[:, sl])

            # ---- projections: qT = Wqk^T Q^T / NF,  kT = Wqk^T K^T --------
            qmt = sb.tile([D, S], BF16, tag="qmt", bufs=2)
            kmt = sb.tile([D, S], BF16, tag="kmt", bufs=2)
            for src, dst, scale in ((qtb, qmt, 1.0 / NF), (ktb, kmt, 1.0)):
                for c in range(2):
                    p = ps.tile(
                        [D, S // 2], F32, tag="big", bufs=2,
                        name=f"proj_ps_{h}_{dst.tensor.name}_{c}",
                    )
                    for c2 in range(2):
                        nc.tensor.matmul(
                            p[:, c2 * 512 : (c2 + 1) * 512],
                            wqkb[:],
                            src[:, c * 1024 + c2 * 512 : c * 1024 + (c2 + 1) * 512],
                        )
                    nc.scalar.activation(
                        dst[:, c * 1024 : (c + 1) * 1024], p[:], AF.Copy, scale=scale
                    )

            # ---- v chunks in natural [s,e] layout: v = V_raw @ Wv ---------
            vsb = sb.tile([D, NB * D], BF16, tag="vsb", bufs=2)
            for c in range(2):
                vp = ps.tile([D, S // 2], F32, tag="big", bufs=2, name=f"vp_ps_{h}_{c}")
                for j in range(8):
                    nc.tensor.matmul(
                        vp[:, j * 128 : (j + 1) * 128],
                        vtb[:, (c * 8 + j) * 128 : (c * 8 + j + 1) * 128],
                        wvb[:],
                    )
                nc.vector.tensor_copy(vsb[:, c * 1024 : (c + 1) * 1024], vp[:])

            # ---- pass 1: scoresT -> exp(bf16), left-padded with zeros -----
            # expt_j buffer holds PAD zero columns then the w real columns,
            # so later N=512 reads spanning "before the diagonal" see zeros.
            expts = []
            for j in range(NB):
                w = S - j * 128  # sq columns j*128 .. S
                expt = sb.tile(
                    [D, PAD + w], BF16, tag=f"expt{j}", bufs=2, name=f"expt_h{h}_{j}"
                )
                nc.gpsimd.memset(expt[:, 0:PAD], 0.0)
                nhalf = (w + 1023) // 1024
                for c in range(nhalf):
                    lo = c * 1024
                    cw = min(1024, w - lo)
                    sc_ps = ps.tile(
                        [D, cw], F32, tag="big", bufs=2, name=f"sc_ps_h{h}_{j}_{c}"
                    )
                    for c2 in range(0, cw, 512):
                        ce = min(c2 + 512, cw)
                        first = c == 0 and c2 == 0
                        nc.tensor.matmul(
                            sc_ps[:, c2:ce],
                            kmt[:, j * 128 : (j + 1) * 128],
                            qmt[:, j * 128 + lo + c2 : j * 128 + lo + ce],
                            start=True,
                            stop=not first,
                        )
                        if first:
                            # causal mask on diagonal block via PE accumulate
                            nc.tensor.matmul(
                                sc_ps[:, 0:128],
                                identb[:],
                                maskb[:],
                                start=False,
                                stop=True,
                            )
                    nc.scalar.activation(
                        expt[:, PAD + lo : PAD + lo + cw], sc_ps[:], AF.Exp
                    )
                expts.append(expt)

            # ---- softmax sums: ones-stationary N=512 row-sums -------------
            recip_ps = ps.tile([D, NB], F32, tag="recipps", name=f"recip_ps_{h}")
            for c in range(4):
                srow = ps.tile([1, 512], F32, tag="sumsrow", name=f"srow_{h}_{c}")
                njc = 4 * c + 4  # j = 0 .. 4c+3 contribute to this chunk
                for j in range(njc):
                    nc.tensor.matmul(
                        srow[:],
                        onesb[:],
                        expts[j][:, PAD + 512 * c - 128 * j : PAD + 512 * c - 128 * j + 512],
                        start=(j == 0),
                        stop=(j == njc - 1),
                    )
                srow_sb = sb.tile([1, 512], F32, tag="srow_sb", bufs=2)
                nc.vector.tensor_copy(srow_sb[:], srow[:])
                for s4 in range(4):
                    i = c * 4 + s4
                    # [1,128] row -> [128,1] column via K=1 matmul
                    nc.tensor.matmul(
                        recip_ps[:, i : i + 1],
                        srow_sb[0:1, s4 * 128 : (s4 + 1) * 128],
                        onesf[:],
                    )
            recip = sb.tile([D, NB], F32, tag="recip", bufs=2)
            nc.vector.reciprocal(recip[:], recip_ps[:])

            # ---- pass 2: PV accumulation, transpose, normalize, store -----
            for i4 in range(NB // 4):
                outt_ps = ps.tile([D, 512], F32, tag="outt", name=f"outt_{h}_{i4}")
                njc = 4 * i4 + 4
                for j in range(njc):
                    nc.tensor.matmul(
                        outt_ps[:],
                        vsb[:, j * 128 : (j + 1) * 128],
                        expts[j][:, PAD + 512 * i4 - 128 * j : PAD + 512 * i4 - 128 * j + 512],
                        start=(j == 0),
                        stop=(j == njc - 1),
                    )
                outt_sb = sb.tile([D, 512], F32, tag="outt_sb", bufs=2)
                nc.vector.tensor_copy(outt_sb[:], outt_ps[:])
                otr_ps = ps.tile([D, 512], F32, tag="otr", name=f"otr_{h}_{i4}")
                osb = sb.tile([D, 512], F32, tag="osb", bufs=2)
                for s4 in range(4):
                    i = i4 * 4 + s4
                    sl = slice(s4 * 128, (s4 + 1) * 128)
                    nc.tensor.transpose(otr_ps[:, sl], outt_sb[:, sl], identf[:])
                    nc.vector.tensor_scalar_mul(
                        osb[:, sl], otr_ps[:, sl], recip[:, i : i + 1]
                    )
                nc.sync.dma_start(
                    out_d[h, i4 * 512 : (i4 + 1) * 512, :].rearrange(
                        "(b s) e -> s b e", b=4
                    ),
                    osb[:].rearrange("p (b e) -> p b e", b=4),
                )

    nc.compile()
    return nc


_NC_CACHE = None


def _get_program():
    global _NC_CACHE
    if _NC_CACHE is None:
        _NC_CACHE = build_program()
    return _NC_CACHE


def make_in_maps(query_layer, key_layer, value_layer, svd_qk, svd_v):
    qt = np.ascontiguousarray(query_layer[:, 0].transpose(1, 2, 0))
    kt = np.ascontiguousarray(key_layer[:, 0].transpose(1, 2, 0))
    vt = np.ascontiguousarray(value_layer[:, 0].transpose(1, 2, 0))
    svd_qk = np.ascontiguousarray(svd_qk, dtype=np.float32)
    svd_v = np.ascontiguousarray(svd_v, dtype=np.float32)

    identf = np.eye(D, dtype=np.float32)
    identb = np.eye(D, dtype=ml_dtypes.bfloat16)
    r = np.arange(D)
    maskb = np.where(r[:, None] > r[None, :], NEG, 0.0).astype(ml_dtypes.bfloat16)
    onesb = np.ones((D, 1), dtype=ml_dtypes.bfloat16)

    in_maps = []
    for c in range(NCORES):
        hs = slice(c * HPC, (c + 1) * HPC)
        in_maps.append(
            {
                "qt": qt[hs],
                "kt": kt[hs],
                "vt": vt[hs],
                "wqk": svd_qk[hs],
                "wv": svd_v[hs],
                "identf": identf,
                "identb": identb,
                "maskb": maskb,
                "onesb": onesb,
                "onesf": np.ones((1, 1), dtype=np.float32),
            }
        )
    return in_maps


def assemble_output(results):
    out = np.empty((S, B, H * D), dtype=np.float32)
    for c in range(NCORES):
        o = results[c]["out"]  # [HPC, S, D]
        for hl in range(HPC):
            h = c * HPC + hl
            out[:, 0, h * D : (h + 1) * D] = o[hl]
    return out


def kernel(query_layer, key_layer, value_layer, attention_mask, svd_qk, svd_v):
    nc = _get_program()
    in_maps = make_in_maps(query_layer, key_layer, value_layer, svd_qk, svd_v)
    res = run_bass_kernel_spmd(nc, in_maps, list(range(NCORES))).results
    return assemble_output(res)



# revision 3
# speedup vs baseline: 1.4537x; 1.4537x over previous
"""Trainium2 Bass kernel for nn_CoreAttention (S=2048, B=1, H=16, D=128).

Sharding: 16 heads across 8 NeuronCores (2 heads/core, tensor parallel).

Per head (everything stays feature-major; host supplies bf16 Q^T/NF,
K^T, V^T per head):
    qmt    = Wqk^T (Q^T/NF)              (bf16 PE, fp32 PSUM, DVE cast)
    kmt    = Wqk^T K^T                   (bf16 PE)
    vaug_j = [V_j Wv | ones]             (bf16, [s,e] layout + ones col)
    per key-block i:
      scoresT[k,q] = kmt_i^T @ qmt       (bf16 PE; causal: only q >= i*128)
      scoresT += causal mask on diagonal (PE accumulate of -1e4 tile)
      expT_i = exp(scoresT)              (ACT, PSUM->SBUF bf16)
    per query-block i (interleaved one step behind scores):
      ctx_aug[q,0:129] = sum_j expT_j(q-block i)^T @ vaug_j   (PE)
        -> cols 0:128 = unnormalized context, col 128 = softmax denom
      ctx = ctx_aug[:,0:128] * (1/ctx_aug[:,128])  (DVE recip + scalar mul)
      DMA ctx -> out[q-block i] (already [q, e] natural layout)

The stationary operand of the PV matmul is the exp'd score block, so the
ones column of vaug yields the softmax denominator for free and the
output lands in [q, e] layout -- no separate row-sum pass, no PE
transposes.

exp() runs without max-subtraction: scores are ~N(0,1) (the reference
normalizes by sqrt(128)), so exp never overflows and matches the
reference's masked softmax to rounding error.
"""

import sys
from contextlib import ExitStack

import numpy as np

for _p in ("/opt/trn_rl_repo",):
    if _p not in sys.path:
        sys.path.insert(0, _p)

import ml_dtypes
import concourse.bass as bass
import concourse.tile as tile
from concourse import bacc, mybir
from concourse.bass_utils import run_bass_kernel_spmd

S, B, H, D = 2048, 1, 16, 128
HPC = 2  # heads per core
NCORES = 8
NB = S // 128  # 16 seq blocks of 128
NF = float(np.sqrt(2048.0 / 16.0))  # NORM_FACTOR
NEG = -10000.0
VSTRIDE = 132  # per-key-block stride in vaug: 128 v cols + 1 ones + 3 pad

F32 = mybir.dt.float32
BF16 = mybir.dt.bfloat16
AF = mybir.ActivationFunctionType


def build_program() -> bass.Bass:
    nc = bacc.Bacc(
        "TRN2", target_bir_lowering=False, debug=False, num_devices=NCORES
    )

    qt_d = nc.dram_tensor("qt", [HPC, D, S], BF16, kind="ExternalInput")
    kt_d = nc.dram_tensor("kt", [HPC, D, S], BF16, kind="ExternalInput")
    vt_d = nc.dram_tensor("vt", [HPC, D, S], BF16, kind="ExternalInput")
    wqk_d = nc.dram_tensor("wqk", [HPC, D, D], BF16, kind="ExternalInput")
    wv_d = nc.dram_tensor("wv", [HPC, D, D], BF16, kind="ExternalInput")
    identb_d = nc.dram_tensor("identb", [D, D], BF16, kind="ExternalInput")
    maskb_d = nc.dram_tensor("maskb", [D, D], BF16, kind="ExternalInput")
    out_d = nc.dram_tensor("out", [HPC, S, D], F32, kind="ExternalOutput")

    with tile.TileContext(nc) as tc, ExitStack() as ctx:
        cpool = ctx.enter_context(tc.tile_pool(name="const", bufs=1))
        sb = ctx.enter_context(tc.tile_pool(name="sb", bufs=1))
        ps = ctx.enter_context(tc.tile_pool(name="ps", bufs=1, space="PSUM"))

        identb = cpool.tile([D, D], BF16)
        nc.sync.dma_start(identb[:], identb_d[:])
        maskb = cpool.tile([D, D], BF16)
        nc.sync.dma_start(maskb[:], maskb_d[:])

        wqkb, wvb, qtb, ktb, vtb = {}, {}, {}, {}, {}
        qmt, kmt, vaug, expt = {}, {}, {}, {}

        # ---- input DMA (weights first: tiny, unblock projections) --------
        for h in range(HPC):
            wqkb[h] = sb.tile([D, D], BF16, tag="wqk", bufs=2, name=f"wqkb{h}")
            nc.sync.dma_start(wqkb[h][:], wqk_d[h])
            wvb[h] = sb.tile([D, D], BF16, tag="wv", bufs=2, name=f"wvb{h}")
            nc.sync.dma_start(wvb[h][:], wv_d[h])
            qtb[h] = sb.tile([D, S], BF16, tag="qtb", bufs=2, name=f"qtb{h}")
            ktb[h] = sb.tile([D, S], BF16, tag="ktb", bufs=2, name=f"ktb{h}")
            vtb[h] = sb.tile([D, S], BF16, tag="vtb", bufs=2, name=f"vtb{h}")
            for raw, dr in ((qtb[h], qt_d), (ktb[h], kt_d), (vtb[h], vt_d)):
                for c in range(2):
                    sl = slice(c * 1024, (c + 1) * 1024)
                    nc.sync.dma_start(raw[:, sl], dr[h][:, sl])

        # ---- projections ------------------------------------------------
        for h in range(HPC):
            qmt[h] = sb.tile([D, S], BF16, tag="qmt", bufs=2, name=f"qmt{h}")
            kmt[h] = sb.tile([D, S], BF16, tag="kmt", bufs=2, name=f"kmt{h}")
            for src, dst in ((qtb[h], qmt[h]), (ktb[h], kmt[h])):
                for c in range(2):
                    p = ps.tile(
                        [D, 1024], F32, tag="mm1024", bufs=2,
                        name=f"proj_{h}_{dst.tensor.name}_{c}",
                    )
                    for c2 in range(2):
                        nc.tensor.matmul(
                            p[:, c2 * 512 : (c2 + 1) * 512],
                            wqkb[h][:],
                            src[:, c * 1024 + c2 * 512 : c * 1024 + (c2 + 1) * 512],
                        )
                    nc.vector.tensor_copy(dst[:, c * 1024 : (c + 1) * 1024], p[:])

            # v chunks in [s, e] layout + ones column at col 128 of each
            vaug[h] = sb.tile([D, NB * VSTRIDE], BF16, tag="vaug", bufs=2, name=f"vaug{h}")
            nc.gpsimd.memset(vaug[h][:], 1.0)
            for c in range(2):
                vp = ps.tile([D, 1024], F32, tag="mm1024", bufs=2, name=f"vp_{h}_{c}")
                for j8 in range(8):
                    j = c * 8 + j8
                    nc.tensor.matmul(
                        vp[:, j8 * 128 : (j8 + 1) * 128],
                        vtb[h][:, j * 128 : (j + 1) * 128],
                        wvb[h][:],
                    )
                for j8 in range(8):
                    j = c * 8 + j8
                    nc.vector.tensor_copy(
                        vaug[h][:, j * VSTRIDE : j * VSTRIDE + 128],
                        vp[:, j8 * 128 : (j8 + 1) * 128],
                    )

        # ---- PV for one query block: ctx + softmax denom in one pass ----
        def emit_pv(h, i):
            pvp = ps.tile([D, 512], F32, tag="pv", bufs=2, name=f"pv_{h}_{i}")
            for j in range(i + 1):
                nc.tensor.matmul(
                    pvp[:, 0:129],
                    expt[(h, j)][:, (i - j) * 128 : (i - j) * 128 + 128],
                    vaug[h][:, j * VSTRIDE : j * VSTRIDE + 129],
                    start=(j == 0),
                    stop=(j == i),
                )
            r = sb.tile([D, 1], F32, tag="rec", bufs=4, name=f"rec_{h}_{i}")
            nc.vector.reciprocal(r[:], pvp[:, 128:129])
            o = sb.tile([D, D], F32, tag="osb", bufs=4, name=f"osb_{h}_{i}")
            nc.vector.tensor_scalar_mul(o[:], pvp[:, 0:128], r[:])
            nc.sync.dma_start(out_d[h, i * 128 : (i + 1) * 128, :], o[:])

        # ---- interleaved scores/exp (block i) and PV (block i-1) --------
        for i in range(NB):
            w = S - i * 128  # q columns i*128 .. S for key block i
            for h in range(HPC):
                expt[(h, i)] = sb.tile(
                    [D, w], BF16, tag=f"expt{i}", bufs=2, name=f"expt_h{h}_{i}"
                )
                for c in range((w + 1023) // 1024):
                    lo = c * 1024
                    cw = min(1024, w - lo)
                    scp = ps.tile(
                        [D, cw], F32, tag="mm1024", bufs=2, name=f"sc_{h}_{i}_{c}"
                    )
                    for c2 in range(0, cw, 512):
                        ce = min(c2 + 512, cw)
                        first = c == 0 and c2 == 0
                        nc.tensor.matmul(
                            scp[:, c2:ce],
                            kmt[h][:, i * 128 : (i + 1) * 128],
                            qmt[h][:, i * 128 + lo + c2 : i * 128 + lo + ce],
                            start=True,
                            stop=not first,
                        )
                        if first:
                            # causal mask on diagonal block via PE accumulate
                            nc.tensor.matmul(
                                scp[:, 0:128],
                                identb[:],
                                maskb[:],
                                start=False,
                                stop=True,
                            )
                    nc.scalar.activation(
                        expt[(h, i)][:, lo : lo + cw], scp[:], AF.Exp
                    )
            if i >= 1:
                for h in range(HPC):
                    emit_pv(h, i - 1)
        for h in range(HPC):
            emit_pv(h, NB - 1)

    nc.compile()
    return nc


_NC_CACHE = None


def _get_program():
    global _NC_CACHE
    if _NC_CACHE is None:
        _NC_CACHE = build_program()
    return _NC_CACHE


def make_in_maps(query_layer, key_layer, value_layer, svd_qk, svd_v):
    qt = (query_layer[:, 0].transpose(1, 2, 0) / NF).astype(ml_dtypes.bfloat16)
    kt = key_layer[:, 0].transpose(1, 2, 0).astype(ml_dtypes.bfloat16)
    vt = value_layer[:, 0].transpose(1, 2, 0).astype(ml_dtypes.bfloat16)
    wqk = np.asarray(svd_qk, dtype=np.float32).astype(ml_dtypes.bfloat16)
    wv = np.asarray(svd_v, dtype=np.float32).astype(ml_dtypes.bfloat16)

    identb = np.eye(D, dtype=ml_dtypes.bfloat16)
    r = np.arange(D)
    maskb = np.where(r[:, None] > r[None, :], NEG, 0.0).astype(ml_dtypes.bfloat16)

    in_maps = []
    for c in range(NCORES):
        hs = slice(c * HPC, (c + 1) * HPC)
        in_maps.append(
            {
                "qt": np.ascontiguousarray(qt[hs]),
                "kt": np.ascontiguousarray(kt[hs]),
                "vt": np.ascontiguousarray(vt[hs]),
                "wqk": np.ascontiguousarray(wqk[hs]),
                "wv": np.ascontiguousarray(wv[hs]),
                "identb": identb,
                "maskb": maskb,
            }
        )
    return in_maps


def assemble_output(results):
    out = np.empty((S, B, H * D), dtype=np.float32)
    for c in range(NCORES):
        o = results[c]["out"]  # [HPC, S, D]
        for hl in range(HPC):
            h = c * HPC + hl
            out[:, 0, h * D : (h + 1) * D] = o[hl]
    return out


def kernel(query_layer, key_layer, value_layer, attention_mask, svd_qk, svd_v):
    nc = _get_program()
    in_maps = make_in_maps(query_layer, key_layer, value_layer, svd_qk, svd_v)
    res = run_bass_kernel_spmd(nc, in_maps, list(range(NCORES))).results
    return assemble_output(res)


# revision 5
# speedup vs baseline: 1.5791x; 1.0862x over previous
"""Trainium2 Bass kernel for nn_CoreAttention (S=2048, B=1, H=16, D=128).

Sharding: 16 heads across 8 NeuronCores (2 heads/core, tensor parallel).

Per head (everything stays feature-major; host supplies bf16 Q^T/NF,
K^T, V^T per head):
    qmt    = Wqk^T (Q^T/NF)              (bf16 PE, fp32 PSUM, DVE cast)
    kmt    = Wqk^T K^T                   (bf16 PE)
    vaug_j = [V_j Wv | ones]             (bf16, [s,e] layout + ones col)
    per key-block i:
      scoresT[k,q] = kmt_i^T @ qmt       (bf16 PE; causal: only q >= i*128)
      expT_i = exp(scoresT)              (ACT, PSUM->SBUF bf16)
      diag block of expT_i *= 0/1 mask   (GpSimd, zeroes the causal upper)
    per query-block i (interleaved two steps behind scores):
      ctx_aug[q,0:129] = sum_j expT_j(q-block i)^T @ vaug_j   (PE)
        -> cols 0:128 = unnormalized context, col 128 = softmax denom
      ctx = ctx_aug[:,0:128] * (1/ctx_aug[:,128])  (DVE recip + scalar mul)
      DMA ctx -> out, batched 4 query blocks per transfer

The stationary operand of the PV matmul is the exp'd score block, so the
ones column of vaug yields the softmax denominator for free and the
output lands in [q, e] layout -- no separate row-sum pass, no PE
transposes.  PV matmuls are interleaved with the scores matmuls at
instruction granularity so their per-matmul LDWEIGHTS (the PV cadence
limiter) hides under the 512-column score streams.

exp() runs without max-subtraction: scores fit comfortably in bf16/fp32
(max observed exp(score) ~1e4), matching the reference's masked softmax
to rounding error.
"""

import sys
from contextlib import ExitStack

import numpy as np

for _p in ("/opt/trn_rl_repo",):
    if _p not in sys.path:
        sys.path.insert(0, _p)

import ml_dtypes
import concourse.bass as bass
import concourse.tile as tile
from concourse import bacc, mybir
from concourse.bass_utils import run_bass_kernel_spmd

S, B, H, D = 2048, 1, 16, 128
HPC = 2  # heads per core
NCORES = 8
NB = S // 128  # 16 seq blocks of 128
NF = float(np.sqrt(2048.0 / 16.0))  # NORM_FACTOR
VSTRIDE = 132  # per-key-block stride in vaug: 128 v cols + 1 ones + 3 pad
NCONST = 5  # wqk h0, wqk h1, wv h0, wv h1, mask01

F32 = mybir.dt.float32
BF16 = mybir.dt.bfloat16
AF = mybir.ActivationFunctionType


def build_program() -> bass.Bass:
    nc = bacc.Bacc(
        "TRN2", target_bir_lowering=False, debug=False, num_devices=NCORES
    )

    qt_d = nc.dram_tensor("qt", [HPC, D, S], BF16, kind="ExternalInput")
    kt_d = nc.dram_tensor("kt", [HPC, D, S], BF16, kind="ExternalInput")
    vt_d = nc.dram_tensor("vt", [HPC, D, S], BF16, kind="ExternalInput")
    consts_d = nc.dram_tensor("consts", [NCONST, D, D], BF16, kind="ExternalInput")
    out_d = nc.dram_tensor("out", [HPC, S, D], F32, kind="ExternalOutput")

    with tile.TileContext(nc) as tc, ExitStack() as ctx:
        cpool = ctx.enter_context(tc.tile_pool(name="const", bufs=1))
        sb = ctx.enter_context(tc.tile_pool(name="sb", bufs=1))
        ps = ctx.enter_context(tc.tile_pool(name="ps", bufs=1, space="PSUM"))

        consts = cpool.tile([D, NCONST * D], BF16)
        nc.sync.dma_start(
            consts[:].rearrange("p (n c) -> p n c", n=NCONST),
            consts_d[:].rearrange("n p c -> p n c"),
        )
        wqkb = {h: consts[:, h * D : (h + 1) * D] for h in range(HPC)}
        wvb = {h: consts[:, (2 + h) * D : (3 + h) * D] for h in range(HPC)}
        mask01 = consts[:, 4 * D : 5 * D]

        qtb, ktb, vtb, qmt, kmt, vaug, expt, osb = {}, {}, {}, {}, {}, {}, {}, {}

        # ---- input DMA: one transfer per tensor per head, q/k first ------
        for h in range(HPC):
            qtb[h] = sb.tile([D, S], BF16, tag="qtb", bufs=2, name=f"qtb{h}")
            ktb[h] = sb.tile([D, S], BF16, tag="ktb", bufs=2, name=f"ktb{h}")
            nc.sync.dma_start(qtb[h][:], qt_d[h])
            nc.sync.dma_start(ktb[h][:], kt_d[h])
        for h in range(HPC):
            vtb[h] = sb.tile([D, S], BF16, tag="vtb", bufs=2, name=f"vtb{h}")
            nc.sync.dma_start(vtb[h][:], vt_d[h])

        # vaug ones backdrop (cols j*VSTRIDE+128.. stay 1.0 after v copies)
        for h in range(HPC):
            vaug[h] = sb.tile(
                [D, NB * VSTRIDE], BF16, tag="vaug", bufs=2, name=f"vaug{h}"
            )
            nc.gpsimd.memset(vaug[h][:], 1.0)

        # ---- q/k projections, finest-need-first order --------------------
        for h in range(HPC):
            qmt[h] = sb.tile([D, S], BF16, tag="qmt", bufs=2, name=f"qmt{h}")
            kmt[h] = sb.tile([D, S], BF16, tag="kmt", bufs=2, name=f"kmt{h}")
        for c in range(2):
            for h in range(HPC):
                for src, dst in ((qtb[h], qmt[h]), (ktb[h], kmt[h])):
                    p = ps.tile(
                        [D, 1024], F32, tag="mm1024", bufs=2,
                        name=f"proj_{h}_{dst.tensor.name}_{c}",
                    )
                    for c2 in range(2):
                        nc.tensor.matmul(
                            p[:, c2 * 512 : (c2 + 1) * 512],
                            wqkb[h],
                            src[:, c * 1024 + c2 * 512 : c * 1024 + (c2 + 1) * 512],
                        )
                    nc.vector.tensor_copy(dst[:, c * 1024 : (c + 1) * 1024], p[:])

        # ---- thunk builders ---------------------------------------------
        def scores_thunks(h, i):
            """PE thunks for key block i of head h; ACT exp and the gpsimd
            diag-mask multiply are bundled after the last MM of each chunk."""
            w = S - i * 128
            expt[(h, i)] = sb.tile(
                [D, w], BF16, tag=f"expt{i}", bufs=2, name=f"expt_h{h}_{i}"
            )
            thunks = []
            for c in range((w + 1023) // 1024):
                lo = c * 1024
                cw = min(1024, w - lo)
                mms = [(c2, min(c2 + 512, cw)) for c2 in range(0, cw, 512)]

                def chunk_thunk(h=h, i=i, c=c, lo=lo, cw=cw, mms=mms):
                    scp = ps.tile(
                        [D, cw], F32, tag="mm1024", bufs=2, name=f"sc_{h}_{i}_{c}"
                    )
                    for c2, ce in mms:
                        nc.tensor.matmul(
                            scp[:, c2:ce],
                            kmt[h][:, i * 128 : (i + 1) * 128],
                            qmt[h][:, i * 128 + lo + c2 : i * 128 + lo + ce],
                            skip_group_check=True,
                        )
                    nc.scalar.activation(
                        expt[(h, i)][:, lo : lo + cw], scp[:], AF.Exp
                    )
                    if c == 0:
                        dg = expt[(h, i)][:, 0:128]
                        nc.gpsimd.tensor_mul(dg, dg, mask01)

                thunks.append(chunk_thunk)
            return thunks

        def pv_thunks(h, i):
            """PE thunks for the PV accumulation of query block i of head h;
            normalize + batched output DMA bundled after the last pair."""
            pvp = ps.tile([D, 512], F32, tag="pv", bufs=4, name=f"pv_{h}_{i}")
            thunks = []
            for j in range(i + 1):

                def pair_thunk(h=h, i=i, j=j, pvp=pvp):
                    nc.tensor.matmul(
                        pvp[:, 0:129],
                        expt[(h, j)][:, (i - j) * 128 : (i - j) * 128 + 128],
                        vaug[h][:, j * VSTRIDE : j * VSTRIDE + 129],
                        start=(j == 0),
                        stop=(j == i),
                        skip_group_check=True,
                    )
                    if j == i:
                        r = sb.tile([D, 1], F32, tag="rec", bufs=4, name=f"rec_{h}_{i}")
                        nc.vector.reciprocal(r[:], pvp[:, 128:129])
                        if i % 4 == 0:
                            osb[h] = sb.tile(
                                [D, 512], F32, tag="osb", bufs=4, name=f"osb_{h}_{i}"
                            )
                        nc.vector.tensor_scalar_mul(
                            osb[h][:, (i % 4) * 128 : (i % 4 + 1) * 128],
                            pvp[:, 0:128],
                            r[:],
                        )
                        if i % 4 == 3:
                            g = i // 4
                            nc.sync.dma_start(
                                out_d[h, g * 512 : (g + 1) * 512, :].rearrange(
                                    "(b s) e -> s b e", b=4
                                ),
                                osb[h][:].rearrange("p (b e) -> p b e", b=4),
                            )

                thunks.append(pair_thunk)
            return thunks

        def vproj_thunks(h):
            """PE thunks for the v projection (LDW-heavy N=128 pairs);
            batched strided casts into vaug bundled after each 8-block run."""
            thunks = []
            for c in range(2):

                def head_thunk(h=h, c=c, first=True):
                    pass

                vp_holder = {}

                def mk(h=h, c=c, j8=0, vp_holder=vp_holder):
                    def t():
                        if j8 == 0:
                            vp_holder["t"] = ps.tile(
                                [D, 1024], F32, tag="mm1024", bufs=2,
                                name=f"vp_{h}_{c}",
                            )
                        vp = vp_holder["t"]
                        j = c * 8 + j8
                        nc.tensor.matmul(
                            vp[:, j8 * 128 : (j8 + 1) * 128],
                            vtb[h][:, j * 128 : (j + 1) * 128],
                            wvb[h],
                            skip_group_check=True,
                        )
                        if j8 == 7:
                            dst = (
                                vaug[h][:, c * 8 * VSTRIDE : (c + 1) * 8 * VSTRIDE]
                                .rearrange("p (j x) -> p j x", x=VSTRIDE)[:, :, 0:128]
                            )
                            src = vp[:].rearrange("p (j x) -> p j x", x=128)
                            nc.vector.tensor_copy(dst, src)

                    return t

                for j8 in range(8):
                    thunks.append(mk(h=h, c=c, j8=j8))
            return thunks

        def interleave(primary, secondary):
            """Emit primary (score) thunks spread evenly through the
            secondary (PV) thunk stream."""
            if not primary:
                for t in secondary:
                    t()
                return
            step = max(1, (len(secondary) + len(primary) - 1) // len(primary))
            si = 0
            for pt in primary:
                pt()
                for _ in range(step):
                    if si < len(secondary):
                        secondary[si]()
                        si += 1
            while si < len(secondary):
                secondary[si]()
                si += 1

        # ---- main interleaved loop --------------------------------------
        for i in range(NB):
            sc = scores_thunks(0, i) + scores_thunks(1, i)
            if i == 1:
                other = vproj_thunks(0) + vproj_thunks(1)
            elif i >= 2:
                other = pv_thunks(0, i - 2) + pv_thunks(1, i - 2)
            else:
                other = []
            interleave(sc, other)
        for i in (NB - 2, NB - 1):
            for t in pv_thunks(0, i) + pv_thunks(1, i):
                t()

    nc.compile()
    return nc


_NC_CACHE = None


def _get_program():
    global _NC_CACHE
    if _NC_CACHE is None:
        _NC_CACHE = build_program()
    return _NC_CACHE


def make_in_maps(query_layer, key_layer, value_layer, svd_qk, svd_v):
    qt = (query_layer[:, 0].transpose(1, 2, 0) / NF).astype(ml_dtypes.bfloat16)
    kt = key_layer[:, 0].transpose(1, 2, 0).astype(ml_dtypes.bfloat16)
    vt = value_layer[:, 0].transpose(1, 2, 0).astype(ml_dtypes.bfloat16)
    wqk = np.asarray(svd_qk, dtype=np.float32).astype(ml_dtypes.bfloat16)
    wv = np.asarray(svd_v, dtype=np.float32).astype(ml_dtypes.bfloat16)

    r = np.arange(D)
    mask01 = (r[:, None] <= r[None, :]).astype(ml_dtypes.bfloat16)

    in_maps = []
    for c in range(NCORES):
        hs = slice(c * HPC, c * HPC + HPC)
        consts = np.stack(
            [wqk[c * HPC], wqk[c * HPC + 1], wv[c * HPC], wv[c * HPC + 1], mask01]
        )
        in_maps.append(
            {
                "qt": np.ascontiguousarray(qt[hs]),
                "kt": np.ascontiguousarray(kt[hs]),
                "vt": np.ascontiguousarray(vt[hs]),
                "consts": consts,
            }
        )
    return in_maps


def assemble_output(results):
    out = np.empty((S, B, H * D), dtype=np.float32)
    for c in range(NCORES):
        o = results[c]["out"]  # [HPC, S, D]
        for hl in range(HPC):
            h = c * HPC + hl
            out[:, 0, h * D : (h + 1) * D] = o[hl]
    return out


def kernel(query_layer, key_layer, value_layer, attention_mask, svd_qk, svd_v):
    nc = _get_program()
    in_maps = make_in_maps(query_layer, key_layer, value_layer, svd_qk, svd_v)
    res = run_bass_kernel_spmd(nc, in_maps, list(range(NCORES))).results
    return assemble_output(res)


# revision 9
# speedup vs baseline: 1.6336x; 1.0345x over previous
"""Trainium2 Bass kernel for nn_CoreAttention (S=2048, B=1, H=16, D=128).

Sharding: 16 heads across 8 NeuronCores (2 heads/core, tensor parallel).

Per head (everything stays feature-major; host supplies bf16 Q^T/NF,
K^T, V^T per head):
    qmt    = Wqk^T (Q^T/NF)              (bf16 PE, fp32 PSUM, DVE cast)
    kmt    = Wqk^T K^T                   (bf16 PE)
    vaug_j = [V_j Wv | ones]             (bf16, [s,e] layout + ones col)
    per key-block i:
      scoresT[k,q] = kmt_i^T @ qmt       (bf16 PE; causal: only q >= i*128)
      expT_i = exp(scoresT)              (ACT, PSUM->SBUF bf16)
      diag block of expT_i *= 0/1 mask   (GpSimd, zeroes the causal upper)
    per query-block i (interleaved two steps behind scores):
      ctx_aug[q,0:129] = sum_j expT_j(q-block i)^T @ vaug_j   (PE)
        -> cols 0:128 = unnormalized context, col 128 = softmax denom
      ctx = ctx_aug[:,0:128] * (1/ctx_aug[:,128])  (DVE recip + scalar mul)
      DMA ctx -> out, batched 4 query blocks per transfer

The stationary operand of the PV matmul is the exp'd score block, so the
ones column of vaug yields the softmax denominator for free and the
output lands in [q, e] layout -- no separate row-sum pass, no PE
transposes.  PV matmuls are interleaved with the scores matmuls at
instruction granularity so their per-matmul LDWEIGHTS (the PV cadence
limiter) hides under the 512-column score streams.

exp() runs without max-subtraction: scores fit comfortably in bf16/fp32
(max observed exp(score) ~1e4), matching the reference's masked softmax
to rounding error.
"""

import sys
from contextlib import ExitStack

import numpy as np

for _p in ("/opt/trn_rl_repo",):
    if _p not in sys.path:
        sys.path.insert(0, _p)

import ml_dtypes
import concourse.bass as bass
import concourse.tile as tile
from concourse import bacc, mybir
from concourse.bass_utils import run_bass_kernel_spmd

S, B, H, D = 2048, 1, 16, 128
HPC = 2  # heads per core
NCORES = 8
NB = S // 128  # 16 seq blocks of 128
NF = float(np.sqrt(2048.0 / 16.0))  # NORM_FACTOR
VSTRIDE = 132  # per-key-block stride in vaug: 128 v cols + 1 ones + 3 pad
NCONST = 5  # wqk h0, wqk h1, wv h0, wv h1, mask01

F32 = mybir.dt.float32
BF16 = mybir.dt.bfloat16
AF = mybir.ActivationFunctionType


def build_program() -> bass.Bass:
    nc = bacc.Bacc(
        "TRN2", target_bir_lowering=False, debug=False, num_devices=NCORES
    )

    qt_d = nc.dram_tensor("qt", [HPC, D, S], BF16, kind="ExternalInput")
    kt_d = nc.dram_tensor("kt", [HPC, D, S], BF16, kind="ExternalInput")
    vt_d = nc.dram_tensor("vt", [HPC, D, S], BF16, kind="ExternalInput")
    consts_d = nc.dram_tensor("consts", [NCONST, D, D], BF16, kind="ExternalInput")
    out_d = nc.dram_tensor("out", [HPC, S, D], F32, kind="ExternalOutput")

    with tile.TileContext(nc) as tc, ExitStack() as ctx:
        cpool = ctx.enter_context(tc.tile_pool(name="const", bufs=1))
        sb = ctx.enter_context(tc.tile_pool(name="sb", bufs=1))
        ps = ctx.enter_context(tc.tile_pool(name="ps", bufs=1, space="PSUM"))

        consts = cpool.tile([D, NCONST * D], BF16)
        nc.sync.dma_start(
            consts[:].rearrange("p (n c) -> p n c", n=NCONST),
            consts_d[:].rearrange("n p c -> p n c"),
        )
        wqkb = {h: consts[:, h * D : (h + 1) * D] for h in range(HPC)}
        wvb = {h: consts[:, (2 + h) * D : (3 + h) * D] for h in range(HPC)}
        mask01 = consts[:, 4 * D : 5 * D]

        qtb, ktb, vtb, qmt, kmt, vaug, expt, osb = {}, {}, {}, {}, {}, {}, {}, {}

        # ---- input DMA: 1024-col chunks, need-first order ----------------
        for h in range(HPC):
            qtb[h] = sb.tile([D, S], BF16, tag="qtb", bufs=2, name=f"qtb{h}")
            ktb[h] = sb.tile([D, S], BF16, tag="ktb", bufs=2, name=f"ktb{h}")
        for c in range(2):
            sl = slice(c * 1024, (c + 1) * 1024)
            for h in range(HPC):
                nc.sync.dma_start(qtb[h][:, sl], qt_d[h][:, sl])
                nc.sync.dma_start(ktb[h][:, sl], kt_d[h][:, sl])
        for h in range(HPC):
            vtb[h] = sb.tile([D, S], BF16, tag="vtb", bufs=2, name=f"vtb{h}")
            nc.sync.dma_start(vtb[h][:], vt_d[h])

        # vaug ones backdrop (cols j*VSTRIDE+128.. stay 1.0 after v copies)
        for h in range(HPC):
            vaug[h] = sb.tile(
                [D, NB * VSTRIDE], BF16, tag="vaug", bufs=2, name=f"vaug{h}"
            )
            nc.gpsimd.memset(vaug[h][:], 1.0)

        # ---- q/k projections; PSUM evacuation split ACT (q) / DVE (k) ----
        for h in range(HPC):
            qmt[h] = sb.tile([D, S], BF16, tag="qmt", bufs=2, name=f"qmt{h}")
            kmt[h] = sb.tile([D, S], BF16, tag="kmt", bufs=2, name=f"kmt{h}")
        for c in range(2):
            for h in range(HPC):
                for src, dst, use_act in (
                    (qtb[h], qmt[h], True),
                    (ktb[h], kmt[h], False),
                ):
                    p = ps.tile(
                        [D, 1024], F32, tag="mm1024", bufs=2,
                        name=f"proj_{h}_{dst.tensor.name}_{c}",
                    )
                    for c2 in range(2):
                        nc.tensor.matmul(
                            p[:, c2 * 512 : (c2 + 1) * 512],
                            wqkb[h],
                            src[:, c * 1024 + c2 * 512 : c * 1024 + (c2 + 1) * 512],
                        )
                    d = dst[:, c * 1024 : (c + 1) * 1024]
                    if use_act:
                        nc.scalar.activation(d, p[:], AF.Copy)
                    else:
                        nc.vector.tensor_copy(d, p[:])

        # ---- thunk builders ---------------------------------------------
        def scores_thunks(h, i):
            """PE thunks for key block i of head h; ACT exp and the gpsimd
            diag-mask multiply are bundled after the last MM of each chunk."""
            w = S - i * 128
            expt[(h, i)] = sb.tile(
                [D, w], BF16, tag=f"expt{i}", bufs=2, name=f"expt_h{h}_{i}"
            )
            thunks = []
            for c in range((w + 1023) // 1024):
                lo = c * 1024
                cw = min(1024, w - lo)
                mms = [(c2, min(c2 + 512, cw)) for c2 in range(0, cw, 512)]

                def chunk_thunk(h=h, i=i, c=c, lo=lo, cw=cw, mms=mms):
                    scp = ps.tile(
                        [D, cw], F32, tag="mm1024", bufs=2, name=f"sc_{h}_{i}_{c}"
                    )
                    for c2, ce in mms:
                        nc.tensor.matmul(
                            scp[:, c2:ce],
                            kmt[h][:, i * 128 : (i + 1) * 128],
                            qmt[h][:, i * 128 + lo + c2 : i * 128 + lo + ce],
                            skip_group_check=True,
                        )
                    nc.scalar.activation(
                        expt[(h, i)][:, lo : lo + cw], scp[:], AF.Exp
                    )
                    if c == 0:
                        dg = expt[(h, i)][:, 0:128]
                        nc.gpsimd.tensor_mul(dg, dg, mask01)

                thunks.append(chunk_thunk)
            return thunks

        def pv_thunks(h, i):
            """PE thunks for the PV accumulation of query block i of head h;
            normalize + batched output DMA bundled after the last pair."""
            pvp = ps.tile([D, 512], F32, tag="pv", bufs=4, name=f"pv_{h}_{i}")
            thunks = []
            for j in range(i + 1):

                def pair_thunk(h=h, i=i, j=j, pvp=pvp):
                    nc.tensor.matmul(
                        pvp[:, 0:129],
                        expt[(h, j)][:, (i - j) * 128 : (i - j) * 128 + 128],
                        vaug[h][:, j * VSTRIDE : j * VSTRIDE + 129],
                        start=(j == 0),
                        stop=(j == i),
                        skip_group_check=True,
                    )
                    if j == i:
                        r = sb.tile([D, 1], F32, tag="rec", bufs=4, name=f"rec_{h}_{i}")
                        nc.vector.reciprocal(r[:], pvp[:, 128:129])
                        if i % 4 == 0:
                            osb[h] = sb.tile(
                                [D, 512], F32, tag="osb", bufs=4, name=f"osb_{h}_{i}"
                            )
                        nc.vector.tensor_scalar_mul(
                            osb[h][:, (i % 4) * 128 : (i % 4 + 1) * 128],
                            pvp[:, 0:128],
                            r[:],
                        )
                        # groups 0-2: one DMA per 4 blocks; last group: per
                        # 2 blocks so the kernel tail isn't one big transfer
                        if i < 12 and i % 4 == 3:
                            g = i // 4
                            nc.sync.dma_start(
                                out_d[h, g * 512 : (g + 1) * 512, :].rearrange(
                                    "(b s) e -> s b e", b=4
                                ),
                                osb[h][:].rearrange("p (b e) -> p b e", b=4),
                            )
                        elif i >= 12 and i % 2 == 1:
                            q0 = (i - 1) * 128
                            o0 = ((i - 1) % 4) * 128
                            nc.sync.dma_start(
                                out_d[h, q0 : q0 + 256, :].rearrange(
                                    "(b s) e -> s b e", b=2
                                ),
                                osb[h][:, o0 : o0 + 256].rearrange(
                                    "p (b e) -> p b e", b=2
                                ),
                            )

                thunks.append(pair_thunk)
            return thunks

        def vproj_thunks(h):
            """PE thunks for the v projection (LDW-heavy N=128 pairs);
            batched strided casts into vaug bundled after each 8-block run."""
            thunks = []
            for c in range(2):

                def head_thunk(h=h, c=c, first=True):
                    pass

                vp_holder = {}

                def mk(h=h, c=c, j8=0, vp_holder=vp_holder):
                    def t():
                        if j8 == 0:
                            vp_holder["t"] = ps.tile(
                                [D, 1024], F32, tag="mm1024", bufs=2,
                                name=f"vp_{h}_{c}",
                            )
                        vp = vp_holder["t"]
                        j = c * 8 + j8
                        nc.tensor.matmul(
                            vp[:, j8 * 128 : (j8 + 1) * 128],
                            vtb[h][:, j * 128 : (j + 1) * 128],
                            wvb[h],
                            skip_group_check=True,
                        )
                        if j8 == 7:
                            dst = (
                                vaug[h][:, c * 8 * VSTRIDE : (c + 1) * 8 * VSTRIDE]
                                .rearrange("p (j x) -> p j x", x=VSTRIDE)[:, :, 0:128]
                            )
                            src = vp[:].rearrange("p (j x) -> p j x", x=128)
                            nc.vector.tensor_copy(dst, src)

                    return t

                for j8 in range(8):
                    thunks.append(mk(h=h, c=c, j8=j8))
            return thunks

        def interleave(primary, secondary):
            """Emit primary (score) thunks spread evenly through the
            secondary (PV) thunk stream."""
            if not primary:
                for t in secondary:
                    t()
                return
            step = max(1, (len(secondary) + len(primary) - 1) // len(primary))
            si = 0
            for pt in primary:
                pt()
                for _ in range(step):
                    if si < len(secondary):
                        secondary[si]()
                        si += 1
            while si < len(secondary):
                secondary[si]()
                si += 1

        # ---- main interleaved loop --------------------------------------
        # PV trails scores by 2 blocks early (ACT latency slack), then by 1
        # late so the un-overlapped tail after the last scores is short.
        pv_next = 0
        for i in range(NB):
            sc = scores_thunks(0, i) + scores_thunks(1, i)
            other = []
            if i == 1:
                other = vproj_thunks(0) + vproj_thunks(1)
            delay = 2 if i < 10 else 1
            while pv_next <= i - delay:
                other += pv_thunks(0, pv_next) + pv_thunks(1, pv_next)
                pv_next += 1
            interleave(sc, other)
        while pv_next < NB:
            for t in pv_thunks(0, pv_next) + pv_thunks(1, pv_next):
                t()
            pv_next += 1

    nc.compile()
    return nc


_NC_CACHE = None


def _get_program():
    global _NC_CACHE
    if _NC_CACHE is None:
        _NC_CACHE = build_program()
    return _NC_CACHE


def make_in_maps(query_layer, key_layer, value_layer, svd_qk, svd_v):
    qt = (query_layer[:, 0].transpose(1, 2, 0) / NF).astype(ml_dtypes.bfloat16)
    kt = key_layer[:, 0].transpose(1, 2, 0).astype(ml_dtypes.bfloat16)
    vt = value_layer[:, 0].transpose(1, 2, 0).astype(ml_dtypes.bfloat16)
    wqk = np.asarray(svd_qk, dtype=np.float32).astype(ml_dtypes.bfloat16)
    wv = np.asarray(svd_v, dtype=np.float32).astype(ml_dtypes.bfloat16)

    r = np.arange(D)
    mask01 = (r[:, None] <= r[None, :]).astype(ml_dtypes.bfloat16)

    in_maps = []
    for c in range(NCORES):
        hs = slice(c * HPC, c * HPC + HPC)
        consts = np.stack(
            [wqk[c * HPC], wqk[c * HPC + 1], wv[c * HPC], wv[c * HPC + 1], mask01]
        )
        in_maps.append(
            {
                "qt": np.ascontiguousarray(qt[hs]),
                "kt": np.ascontiguousarray(kt[hs]),
                "vt": np.ascontiguousarray(vt[hs]),
                "consts": consts,
            }
        )
    return in_maps


def assemble_output(results):
    out = np.empty((S, B, H * D), dtype=np.float32)
    for c in range(NCORES):
        o = results[c]["out"]  # [HPC, S, D]
        for hl in range(HPC):
            h = c * HPC + hl
            out[:, 0, h * D : (h + 1) * D] = o[hl]
    return out


def kernel(query_layer, key_layer, value_layer, attention_mask, svd_qk, svd_v):
    nc = _get_program()
    in_maps = make_in_maps(query_layer, key_layer, value_layer, svd_qk, svd_v)
    res = run_bass_kernel_spmd(nc, in_maps, list(range(NCORES))).results
    return assemble_output(res)
